# revision 9
# baseline (speedup 1.0000x reference)
"""Bahdanau-style attention kernel for Trainium2, 8 NeuronCores, data-parallel over batch.

Computes, per (b, s):
    energy = tanh(dec @ Wd + enc @ We + b_attn)          # [B,S,H]
    att    = energy @ v_w                                 # [B,S]
    att    = where(mask==1, -1e10, att)
    out    = softmax(att, axis=1)

Full shapes: B=64, S=2048, H=1024. Each core takes 8 batches.

Per-core layout (all compute in fp16 on the PE, f32 accumulation):
  - enc rows are loaded [128 rows, 1024 h], cast to fp16, transposed to
    [128 h, rows] blocks with the xbar DMA transpose so the contraction dim
    (h) lands on partitions.
  - main matmul: psum[kout,rows] += We[h,kout].T @ encT[h,rows], per 512-row
    chunk, 8 kout-tiles x 8 h-tiles of MMs.
  - ACT applies tanh(psum + bias[kout]) where bias = dec@Wd + b_attn is
    per-partition (kout), computed once per batch in the setup phase.
  - v_w dot is a rank-1 matmul over the kout partitions -> scores [1, rows].
  - softmax runs on the free dim: exp on ACT, masked multiply + running sum
    with one fused DVE tensor_tensor_reduce, reciprocal + scale, DMA out.
"""
import numpy as np

B, S, H = 64, 2048, 1024
NCORES = 8
BPC = B // NCORES          # batches per core
CHUNK = 512                # rows per chunk
NCH = S // CHUNK           # chunks per batch
NT = CHUNK // 128          # 128-row tiles per chunk
HB = H // 128              # h blocks
KB = H // 128              # kout blocks

_graph_cache = {}


def _build(nb=BPC, nch=NCH):
    import concourse.bass as bass
    import concourse.bacc as bacc
    import concourse.tile as tile
    from concourse import mybir

    F32 = mybir.dt.float32
    F16 = mybir.dt.float16
    I32 = mybir.dt.int32
    AF = mybir.ActivationFunctionType
    ALU = mybir.AluOpType

    nc = bacc.Bacc(trn_type="TRN2", target_bir_lowering=False)

    dec_ext = nc.declare_dram_parameter("dec", [BPC, H], F32, isOutput=False)
    enc_ext = nc.declare_dram_parameter("enc", [BPC, S, H], F32, isOutput=False)
    mask_ext = nc.declare_dram_parameter("mask", [BPC, S], I32, isOutput=False)
    w_ext = nc.declare_dram_parameter("W", [2 * H, H], F32, isOutput=False)
    b_ext = nc.declare_dram_parameter("b", [H], F32, isOutput=False)
    v_ext = nc.declare_dram_parameter("v", [H], F32, isOutput=False)
    out_ext = nc.declare_dram_parameter("out", [BPC, S], F32, isOutput=True)

    with tile.TileContext(nc) as tc:
        with (
            tc.tile_pool(name="weights", bufs=1) as wpool,
            tc.tile_pool(name="consts", bufs=1) as cpool,
            tc.tile_pool(name="wload", bufs=2) as wload,
            tc.tile_pool(name="encload", bufs=2) as epool,
            tc.tile_pool(name="enccast", bufs=2) as bfpool,
            tc.tile_pool(name="enct", bufs=2) as tpool,
            tc.tile_pool(name="energy", bufs=4) as engpool,
            tc.tile_pool(name="rows", bufs=2) as rpool,
            tc.tile_pool(name="psum_mm", bufs=3, space="PSUM") as psum_pool,
            tc.tile_pool(name="psum_vd", bufs=2, space="PSUM") as vd_pool,
        ):
            # ---------------- setup ----------------
            we_f16 = wpool.tile([128, HB, H], F16, tag="we")
            wd_f16 = wpool.tile([128, HB, H], F16, tag="wd")
            for hb in range(HB):
                wt = wload.tile([128, H], F32, tag="wtmp")
                nc.gpsimd.dma_start(out=wt[:], in_=w_ext[H + hb * 128 : H + (hb + 1) * 128, :])
                nc.vector.tensor_copy(we_f16[:, hb, :], wt[:])
                wt2 = wload.tile([128, H], F32, tag="wtmp")
                nc.gpsimd.dma_start(out=wt2[:], in_=w_ext[hb * 128 : (hb + 1) * 128, :])
                nc.vector.tensor_copy(wd_f16[:, hb, :], wt2[:])

            # decT [h -> partitions], per h-block: [128, BPC]
            dect = cpool.tile([128, HB, BPC], F16, tag="dect")
            dtmp = cpool.tile([128, HB, BPC], F32, tag="dectf32")
            for hb in range(HB):
                nc.gpsimd.dma_start(
                    out=dtmp[:, hb, :],
                    in_=dec_ext[:, hb * 128 : (hb + 1) * 128].rearrange("b p -> p b"),
                )
            nc.vector.tensor_copy(dect[:], dtmp[:])

            # b_attn / v_w transposed to [128, HB]
            batt = cpool.tile([128, KB], F32, tag="batt")
            nc.gpsimd.dma_start(out=batt[:], in_=b_ext[:].rearrange("(kb p) -> p kb", p=128))
            vtmpf = cpool.tile([128, KB], F32, tag="vf32")
            nc.gpsimd.dma_start(out=vtmpf[:], in_=v_ext[:].rearrange("(kb p) -> p kb", p=128))
            vt = cpool.tile([128, KB], F16, tag="vt")
            nc.vector.tensor_copy(vt[:], vtmpf[:])

            # bias[kout, b] = (dec @ Wd).T + b_attn, shape [128, KB, BPC]
            bias_sb = cpool.tile([128, KB, BPC], F32, tag="bias")
            for kt in range(KB):
                ps = psum_pool.tile([128, BPC], F32, tag="psetup")
                for hb in range(HB):
                    nc.tensor.matmul(
                        ps[:],
                        wd_f16[:, hb, kt * 128 : (kt + 1) * 128],
                        dect[:, hb, :],
                        start=(hb == 0),
                        stop=(hb == HB - 1),
                    )
                nc.vector.tensor_scalar(bias_sb[:, kt, :], ps[:], batt[:, kt : kt + 1], None, ALU.add)

            # ---------------- main loop ----------------
            for b in range(nb):
                # keep[s] = (mask[b,s] == 0) as f32, on partition 0
                mt = rpool.tile([1, S], I32, tag="mrow")
                nc.gpsimd.dma_start(out=mt[:], in_=mask_ext[b : b + 1, :])
                mtf = rpool.tile([1, S], F32, tag="mrowf")
                nc.vector.tensor_copy(mtf[:], mt[:])
                keep = rpool.tile([1, S], F32, tag="keep")
                nc.vector.tensor_scalar(keep[:], mtf[:], 0.0, None, ALU.is_equal)

                e_row = rpool.tile([1, S], F32, tag="erow")
                zparts = rpool.tile([1, NCH], F32, tag="zparts")
                for c in range(nch):
                    r0 = c * CHUNK
                    enc_f32 = epool.tile([128, NT, H], F32, tag="encf32")
                    nc.gpsimd.dma_start(
                        out=enc_f32[:],
                        in_=enc_ext[b, r0 : r0 + CHUNK, :].rearrange(
                            "(t p) h -> p t h", p=128
                        ),
                    )
                    enc_f16 = bfpool.tile([128, NT, H], F16, tag="encf16")
                    nc.vector.tensor_copy(enc_f16[:], enc_f32[:])

                    # xbar transpose each 128-row tile: [128 r, 1024 h] -> [128 h, hb, 128 r]
                    enct = tpool.tile([128, NT, HB, 128], F16, tag="enct")
                    for t in range(NT):
                        nc.sync.dma_start(
                            out=enct[:, t, :, :], in_=enc_f16[:, t, :], transpose=True
                        )

                    vd = vd_pool.tile([1, CHUNK], F32, tag="vdot")
                    pending = []  # staggered vdot emission to keep PE dense
                    for kt in range(KB):
                        pk = psum_pool.tile([128, CHUNK], F32, tag="pmm")
                        for hb in range(HB):
                            nc.tensor.matmul(
                                pk[:],
                                we_f16[:, hb, kt * 128 : (kt + 1) * 128],
                                enct[:, :, hb, :],
                                start=(hb == 0),
                                stop=(hb == HB - 1),
                            )
                        eng = engpool.tile([128, CHUNK], F16, tag="energy")
                        nc.scalar.activation(
                            eng[:], pk[:], AF.Tanh, bias=bias_sb[:, kt, b : b + 1]
                        )
                        pending.append((kt, eng))
                        if len(pending) >= 2:
                            k0, e0 = pending.pop(0)
                            nc.tensor.matmul(
                                vd[:], vt[:, k0 : k0 + 1], e0[:],
                                start=(k0 == 0), stop=(k0 == KB - 1),
                            )
                    for k0, e0 in pending:
                        nc.tensor.matmul(
                            vd[:], vt[:, k0 : k0 + 1], e0[:],
                            start=(k0 == 0), stop=(k0 == KB - 1),
                        )

                    e_raw = rpool.tile([1, CHUNK], F32, tag="eraw")
                    nc.scalar.activation(e_raw[:], vd[:], AF.Exp)
                    nc.vector.tensor_tensor(
                        e_row[:, r0 : r0 + CHUNK], e_raw[:], keep[:, r0 : r0 + CHUNK], ALU.mult
                    )
                    nc.vector.tensor_reduce(
                        zparts[:, c : c + 1], e_row[:, r0 : r0 + CHUNK],
                        mybir.AxisListType.XYZW, ALU.add,
                    )

                zacc = rpool.tile([1, 1], F32, tag="zacc")
                nc.vector.tensor_reduce(zacc[:], zparts[:], mybir.AxisListType.XYZW, ALU.add)
                zr = rpool.tile([1, 1], F32, tag="zr")
                nc.vector.reciprocal(zr[:], zacc[:])
                out_row = rpool.tile([1, S], F32, tag="orow")
                nc.vector.tensor_scalar(out_row[:], e_row[:], zr[:], None, ALU.mult)
                nc.gpsimd.dma_start(out=out_ext[b : b + 1, :], in_=out_row[:])

    nc.compile()
    return nc


def _get_graph():
    if "nc" not in _graph_cache:
        _graph_cache["nc"] = _build()
    return _graph_cache["nc"]


def kernel(decoder_hidden, encoder_outputs, mask, W_attn, b_attn, v_w):
    from concourse.bass_utils import run_bass_kernel_spmd

    dec = np.asarray(decoder_hidden, dtype=np.float32)
    enc = np.asarray(encoder_outputs, dtype=np.float32)
    msk = np.asarray(mask, dtype=np.int32)
    W = np.asarray(W_attn, dtype=np.float32)
    bb = np.asarray(b_attn, dtype=np.float32)
    vv = np.asarray(v_w, dtype=np.float32)

    nc = _get_graph()
    in_maps = []
    for i in range(NCORES):
        sl = slice(i * BPC, (i + 1) * BPC)
        in_maps.append(
            {"dec": dec[sl], "enc": enc[sl], "mask": msk[sl], "W": W, "b": bb, "v": vv}
        )
    res = run_bass_kernel_spmd(nc, in_maps, core_ids=list(range(NCORES)))
    out = np.concatenate([res.results[i]["out"] for i in range(NCORES)], axis=0)
    return out.astype(np.float32)
